# revision 1
# baseline (speedup 1.0000x reference)
"""AngularPenaltySMLoss (ArcFace) sharded over 8 TRN2 NeuronCores.

Strategy (classification/tensor parallel, classes sharded 8-way):
  - Host: layout prep only — normalize/transpose features and W into
    fp8-friendly range (contraction dim on partitions is a hardware
    matmul requirement) and pad the class dim.
  - Device (per core, SPMD, no collectives needed):
      * stream its W^T shard [512, 12800] f32 from HBM (the memory-bound
        part: 25.6MB/core, 204.8MB total),
      * cast to fp8e4 (x pre-normalized *16 on host, W scaled by 32)
        and matmul with DoubleRow perf mode: psum[b=128, c] tiles,
      * ScalarE: exp(const_scale * psum) with fused free-axis
        accumulation (accum_out) -> per-row partial exp sums,
      * output [128, 8] f32 partial sums per core.
  - Host: gather 8x[1024] partial sums, compute the exact true-class
    logit, the arcface numerator, and the final scalar loss.
"""

import sys

if "/opt/trn_rl_repo" not in sys.path:
    sys.path.insert(0, "/opt/trn_rl_repo")

import numpy as np

S = 64.0
MARGIN = 0.5
EPS = 1e-07
B, D, C = 1024, 512, 100000
NCORES = 8
CSH = C // NCORES            # 12500 real classes per core
CT = 512                     # classes per full c-tile (one PSUM bank fp32)
NT = 25                      # c-tiles per core (last one ragged: 212)
CPAD = NT * CT               # 12800 padded class columns in the wT input
LAST_CT = CSH - (NT - 1) * CT  # 212 real classes in the last tile
NB = B // 128                # 8 row chunks
KT = D // 128                # 4 contraction chunks
WSCALE = 32.0                # fp8 range scaling for W
XSCALE = 16.0                # fp8 range scaling for normalized x

# Supertile grouping: (start_tile, n_tiles). The first group is large
# enough that its ScalarE time covers the second (small) group's DMA;
# after that DMA stays ahead. Each group is <= 4 PSUM banks. (A tiny
# first group measures WORSE despite starting earlier: the extra group's
# instruction overhead and the per-DMA sem/descriptor serialization eat
# the gain.)
GROUPS = [(22, 3), (20, 2), (0, 4), (4, 4), (8, 4), (12, 4), (16, 4)]
NGRP = len(GROUPS)


def _tile_width(t):
    return LAST_CT if t == NT - 1 else CT


_CACHE = {}


def _build_nc():
    from contextlib import ExitStack

    import concourse.bacc as bacc
    import concourse.mybir as mybir
    import concourse.tile as tile
    from concourse.tile_rust import add_dep_helper

    f32 = mybir.dt.float32
    f8 = mybir.dt.float8e4
    AF = mybir.ActivationFunctionType

    nc = bacc.Bacc("TRN2", target_bir_lowering=False, debug=False,
                   num_devices=NCORES)

    xt_ext = nc.dram_tensor("xT", [D, B], f32, kind="ExternalInput")
    wt_ext = nc.dram_tensor("wT", [D, CPAD], f32, kind="ExternalInput")
    out_ext = nc.dram_tensor("out", [128, NB], f32, kind="ExternalOutput")

    # The Tile scheduler breaks priority ties in hash order, which makes
    # the emitted schedule (and ~20% of runtime) depend on PYTHONHASHSEED.
    # Pin each engine's stream to program order with order-only deps.
    _prev = {}

    def _chain(key, bi):
        if key in _prev:
            add_dep_helper(bi.ins, _prev[key].ins, sync=False,
                           reason="deterministic program order")
        _prev[key] = bi
        return bi

    with tile.TileContext(nc) as tc, ExitStack() as ctx:
        const_pool = ctx.enter_context(tc.tile_pool(name="const", bufs=1))
        wtf_pool = ctx.enter_context(tc.tile_pool(name="wtf", bufs=4))
        w8_pool = ctx.enter_context(tc.tile_pool(name="w8", bufs=2))
        psum_pool = ctx.enter_context(
            tc.tile_pool(name="psum", bufs=2, space="PSUM"))

        # Force the ACT exp table load at t=0 (it costs ~2.7us; without
        # this it happens on the critical path at the first real exp).
        warm = const_pool.tile([128, 1], f32)
        nc.gpsimd.memset(warm[:], 0.0)
        nc.scalar.activation(warm[:], warm[:], AF.Exp)

        # Row-chunk-major feature transpose: xt[p, k, b] = x[b, 128k+p].
        # Loaded in per-row-chunk slabs; the first matmul only needs j=0.
        xt_f32 = const_pool.tile([128, KT, B], f32)
        xt8 = const_pool.tile([128, KT, B], f8)
        xt_src = xt_ext.ap().rearrange("(k p) b -> p k b", p=128)

        # All bulk loads go through gpsimd's SWDGE queue 0: strict FIFO
        # order at full bandwidth, so data arrives exactly in consumption
        # order (HWDGE spreads concurrent DMAs over queues, which
        # fair-shares the bandwidth and delays the first-needed tile).
        def load_xt(j, engine):
            sl = slice(j * 128, (j + 1) * 128)
            dma = engine.dma_start(out=xt_f32[:, :, sl], in_=xt_src[:, :, sl])
            if engine is nc.gpsimd:
                _chain("qdma", dma)
            _chain("dve", nc.vector.tensor_copy(xt8[:, :, sl],
                                                xt_f32[:, :, sl]))

        # j=0 rides the (otherwise idle) HWDGE rail and lands before the
        # gpsimd W stream even starts generating descriptors.
        load_xt(0, nc.sync)

        # Throwaway first SWDGE transfer: the first completion on the
        # gpsimd DMA queue pays a one-time ~4-6us init latency (observed
        # constant across layouts); absorb it off the critical path so
        # group0's W semaphore fires promptly.
        dma_warm = const_pool.tile([1, 128], f32)
        _chain("qdma", nc.gpsimd.dma_start(
            out=dma_warm[:], in_=xt_ext.ap()[0:1, 0:128]))

        # Bridge the PE idle window until the first real matmul with
        # throwaway matmuls on the already-cast j0 slab, so the HAM clock
        # gate is warm (2.4 GHz) when group0's matmuls start. Chained
        # first on the PE stream; they finish before group0's W cast.
        warm_ps = psum_pool.tile([128, 4 * CT], f32, tag="ps")
        for r in range(60):
            _chain("pe", nc.tensor.matmul(
                warm_ps[:, :128],
                lhsT=xt8[:, 0:2, 0:128],
                rhs=xt8[:, 0:2, 0:128],
                start=True, stop=True,
                perf_mode=mybir.MatmulPerfMode.DoubleRow,
            ))

        def group_cols(g):
            t0, width = GROUPS[g]
            return sum(_tile_width(t0 + i) for i in range(width))

        def load_w_group(g, w8g, engine):
            """DMA a group's W columns in chunks (512 descriptors each
            regardless of width) and cast each chunk to scaled fp8. The
            first group goes in ONE chunk: each extra dma_start costs a
            serial ~0.9us Q7 descriptor job + ~2us completion latency on
            the ramp's critical path."""
            t0, width = GROUPS[g]
            cols = group_cols(g)
            base = t0 * CT
            cap = 1280 if g == 0 else 1024
            off = 0
            while off < cols:
                cw = min(cap, cols - off)
                wtf = wtf_pool.tile([128, KT, 1280], f32, tag="wtf")
                _chain("qdma", engine.dma_start(
                    out=wtf[:, :, :cw],
                    in_=wt_ext.ap()[:, base + off:base + off + cw]
                    .rearrange("(k p) c -> p k c", p=128)))
                _chain("dve", nc.vector.tensor_scalar_mul(
                    w8g[:, :, off:off + cw], wtf[:, :, :cw], WSCALE))
                off += cw

        # Prologue, interleaved so neither group1's W nor group0's later
        # row-chunks starve: g0 W, xt j1-4, g1 W, xt j5-7. The xT slabs
        # are batched DMAs (one descriptor job each) with per-chunk casts.
        def load_xt_batch(j0, j1):
            sl = slice(j0 * 128, (j1 + 1) * 128)
            _chain("qdma", nc.gpsimd.dma_start(
                out=xt_f32[:, :, sl], in_=xt_src[:, :, sl]))
            for j in range(j0, j1 + 1):
                sj = slice(j * 128, (j + 1) * 128)
                _chain("dve", nc.vector.tensor_copy(xt8[:, :, sj],
                                                    xt_f32[:, :, sj]))

        w8g_pre = []
        for g in range(2):
            w8g = w8_pool.tile([128, KT, 4 * CT], f8, tag="w8g")
            load_w_group(g, w8g, nc.gpsimd)
            w8g_pre.append(w8g)
            load_xt_batch(1, 4) if g == 0 else load_xt_batch(5, 7)

        # Per-(row-chunk, group) partial sums and folded output.
        acc = const_pool.tile([128, NB, NGRP], f32)
        out_s = const_pool.tile([128, NB], f32)

        for g, (t0, width) in enumerate(GROUPS):
            if g < 2:
                w8g = w8g_pre[g]
            else:
                w8g = w8_pool.tile([128, KT, 4 * CT], f8, tag="w8g")
                load_w_group(g, w8g, nc.gpsimd)

            span = group_cols(g)
            for j in range(NB):
                psum = psum_pool.tile([128, 4 * CT], f32, tag="ps")
                for k2 in range(KT // 2):
                    lhsT = xt8[:, 2 * k2:2 * k2 + 2, j * 128:(j + 1) * 128]
                    for i in range(width):
                        cw = _tile_width(t0 + i)
                        _chain("pe", nc.tensor.matmul(
                            psum[:, i * CT:i * CT + cw],
                            lhsT=lhsT,
                            rhs=w8g[:, 2 * k2:2 * k2 + 2,
                                    i * CT:i * CT + cw],
                            start=(k2 == 0),
                            stop=(k2 == KT // 2 - 1),
                            perf_mode=mybir.MatmulPerfMode.DoubleRow,
                        ))
                # exp in place into PSUM: ScalarE sits closer to PSUM and
                # the result values themselves are never read (only the
                # fused accum_out row-sums are).
                _chain("act", nc.scalar.activation(
                    psum[:, :span],
                    psum[:, :span],
                    AF.Exp,
                    scale=S / (WSCALE * XSCALE),
                    accum_out=acc[:, j, g:g + 1],
                ))
                if g == NGRP - 1:
                    # Fold this row-chunk's partial sums while ScalarE is
                    # still streaming the remaining row-chunks; only j=7's
                    # reduce stays on the critical tail.
                    _chain("dve", nc.vector.tensor_reduce(
                        out=out_s[:, j:j + 1], in_=acc[:, j:j + 1, :],
                        axis=mybir.AxisListType.X, op=mybir.AluOpType.add))

        nc.sync.dma_start(out=out_ext.ap(), in_=out_s[:])

    nc.compile()
    return nc


def _host_inputs(features, W):
    """Host-side layout prep: transposes, class padding, per-row scales."""
    x = np.asarray(features, dtype=np.float32)
    Wf = np.asarray(W, dtype=np.float32)

    norms = np.maximum(np.sqrt((x.astype(np.float64) ** 2).sum(1)), 1e-12)
    # normalized, pre-scaled into fp8e4's normal range (same quantization
    # quality as raw x); the exp scale becomes a constant immediate.
    xn16 = (x.astype(np.float64) * (XSCALE / norms)[:, None]).astype(np.float32)
    xT = np.ascontiguousarray(xn16.T)                    # [D, B]

    wT_shards = []
    for m in range(NCORES):
        blk = Wf[m * CSH:(m + 1) * CSH]                  # [12500, 512]
        wt = np.zeros((D, CPAD), dtype=np.float32)
        wt[:, :CSH] = blk.T
        wT_shards.append(wt)
    return xT, wT_shards, norms


def _finish_host(partials, features, W, y_true, norms):
    """Exact scalar assembly from per-core partial exp sums."""
    x64 = np.asarray(features, dtype=np.float64)
    xn = x64 / norms[:, None]
    Wy = np.asarray(W, dtype=np.float64)[np.asarray(y_true)]
    tgt = np.einsum("bd,bd->b", xn, Wy)

    total = np.zeros(B, dtype=np.float64)
    for p in partials:
        total += p.astype(np.float64).T.reshape(B)

    numerator = S * np.cos(np.arccos(np.clip(tgt, -1.0 + EPS, 1.0 - EPS))
                           + MARGIN)
    excl = total - np.exp(S * tgt)
    denom = np.exp(numerator) + excl
    L = numerator - np.log(denom)
    return np.array(-L.mean(), dtype=np.float32)


def _get_nc():
    if "nc" not in _CACHE:
        _CACHE["nc"] = _build_nc()
    return _CACHE["nc"]


def kernel(features, W, y_true):
    from concourse.bass_utils import run_bass_kernel_spmd

    xT, wT_shards, norms = _host_inputs(features, W)
    in_maps = [{"xT": xT, "wT": wT_shards[m]} for m in range(NCORES)]
    nc = _get_nc()
    res = run_bass_kernel_spmd(nc, in_maps, core_ids=list(range(NCORES)))
    partials = [res.results[m]["out"] for m in range(NCORES)]
    return _finish_host(partials, features, W, y_true, norms)

